# revision 2
# baseline (speedup 1.0000x reference)
"""Trainium2 Bass kernel for nn_DeformBasicBlock1 (deformable conv block).

Fused single-invocation design, G-sharded over the 8 channel groups:
core g receives its 8 x-channels plus per-group weights; on-device
AllGather rebuilds full x for the offset convs, the trilinear deform
sampling runs as a dense 5x5x5 shifted-hat expansion contracted on the
PE, and AllReduce / ReduceScatter perform the two cross-group sums.
Both BN layers (training-mode stats), the relu's and the residual are
computed on-device, so one SPMD invocation covers the whole block.
The compiled PJRT executable is cached at module level and NEFFs are
cached on disk keyed by BIR hash.
"""
import hashlib
import json
import os
import shutil
import numpy as np

import concourse.bass as bass
import concourse.mybir as mybir
from concourse.tile import TileContext
import concourse.bass_utils as bass_utils
import concourse.tile_utils as tile_utils

# ---------------------------------------------------------------- tilefix --
# This container's walrus rejects >1 sem-wait per instruction; split extra
# waits onto preceding same-engine NoOps (program order preserves wait
# semantics).  Also cache compiled NEFFs on disk keyed by BIR content hash
# so fresh processes skip the (slow) walrus compile entirely.
_orig_compile_bir_kernel = bass_utils.compile_bir_kernel
_NEFF_CACHE_DIR = os.path.expanduser("~/.cache/bass-neff-cache")


def _split_waits_json(bir_json: bytes) -> bytes:
    j = json.loads(bir_json)
    ctr = 0
    changed = False
    for f in j["functions"]:
        for b in f["blocks"]:
            insts = b["instructions"]
            if not any(
                len((i.get("sync_info") or {}).get("on_wait") or []) > 1
                for i in insts
            ):
                continue
            changed = True
            out = []
            for inst in insts:
                si = inst.get("sync_info")
                if si:
                    ow = si.get("on_wait") or []
                    if len(ow) > 1:
                        for w in ow[:-1]:
                            ctr += 1
                            nop = {
                                "engine": inst["engine"],
                                "ins": [],
                                "outs": [],
                                "name": f"WSPLIT-{ctr}",
                                "opcode": "NoOp",
                                "sync_info": {"on_update": [], "on_wait": [w]},
                            }
                            if "debug" in inst:
                                nop["debug"] = inst["debug"]
                            out.append(nop)
                        si["on_wait"] = [ow[-1]]
                out.append(inst)
            b["instructions"] = out
    return json.dumps(j).encode() if changed else bir_json


def _patched_compile_bir_kernel(bir_json, tmpdir, neff_name="file.neff"):
    if isinstance(bir_json, str):
        bir_json = bir_json.encode()
    bir_json = _split_waits_json(bir_json)
    key = hashlib.sha256(bir_json).hexdigest()
    cached = os.path.join(_NEFF_CACHE_DIR, key + ".neff")
    target = os.path.join(tmpdir, neff_name)
    if os.path.exists(cached):
        shutil.copyfile(cached, target)
        return target
    neff = _orig_compile_bir_kernel(bir_json, tmpdir, neff_name)
    try:
        os.makedirs(_NEFF_CACHE_DIR, exist_ok=True)
        tmp = cached + ".tmp.%d" % os.getpid()
        shutil.copyfile(neff, tmp)
        os.replace(tmp, cached)
    except OSError:
        pass
    return neff


bass_utils.compile_bir_kernel = _patched_compile_bir_kernel
import concourse.bass2jax as _b2j  # noqa: E402

_b2j.compile_bir_kernel = _patched_compile_bir_kernel
try:
    tile_utils.max_sbuf_usage = 204 * 1024
except Exception:
    pass

# --------------------------------------------------- cached PJRT dispatch --
# run_bass_kernel_spmd re-jits (and so re-compiles) the NEFF every call;
# replace its axon execute path with a version that builds the sharded
# jitted callable once per Bass program and reuses it.
import jax  # noqa: E402
from jax.sharding import Mesh, PartitionSpec  # noqa: E402

_orig_run_via_pjrt = _b2j.run_bass_via_pjrt
_PJRT_CACHE = {}


class _CachedSpmd:
    def __init__(self, nc, n_cores):
        _b2j.install_neuronx_cc_hook()
        assert not nc.dbg_callbacks
        # dbg_addr is an unused ExternalInput when there are no callbacks;
        # bind a zero buffer (uint32[1,2], same view run_bass_via_pjrt uses).
        self.dbg_name = nc.dbg_addr.name if nc.dbg_addr is not None else None
        partition_name = (
            nc.partition_id_tensor.name if nc.partition_id_tensor else None
        )
        in_names, out_names, out_avals = [], [], []
        for alloc in nc.m.functions[0].allocations:
            if not isinstance(alloc, mybir.MemoryLocationSet):
                continue
            name = alloc.memorylocations[0].name
            if alloc.kind == "ExternalInput":
                if name != partition_name:
                    in_names.append(name)
            elif alloc.kind == "ExternalOutput":
                out_names.append(name)
                out_avals.append(
                    jax.core.ShapedArray(
                        tuple(alloc.tensor_shape), mybir.dt.np(alloc.dtype)
                    )
                )
        self.in_names = in_names
        self.out_names = out_names
        self.out_avals = out_avals
        self.n_cores = n_cores
        n_params = len(in_names)
        donate = tuple(range(n_params, n_params + len(out_names)))
        all_names = list(in_names + out_names)
        if partition_name is not None:
            all_names.append(partition_name)
        all_names = tuple(all_names)

        def _body(*args):
            operands = list(args)
            if partition_name is not None:
                operands.append(_b2j.partition_id_tensor())
            outs = _b2j._bass_exec_p.bind(
                *operands,
                out_avals=tuple(out_avals),
                in_names=all_names,
                out_names=tuple(out_names),
                lowering_input_output_aliases=(),
                sim_require_finite=True,
                sim_require_nnan=True,
                nc=nc,
            )
            return tuple(outs)

        devices = jax.devices()[:n_cores]
        assert len(devices) == n_cores
        mesh = Mesh(np.asarray(devices), ("core",))
        nspecs = n_params + len(out_names)
        self.fn = jax.jit(
            _b2j.shard_map(
                _body,
                mesh=mesh,
                in_specs=(PartitionSpec("core"),) * nspecs,
                out_specs=(PartitionSpec("core"),) * len(out_names),
                check_rep=False,
            ),
            donate_argnums=donate,
            keep_unused=True,
        )

    def run(self, in_maps):
        n = self.n_cores
        assert len(in_maps) == n
        if self.dbg_name is not None:
            dbg = np.zeros((1, 2), np.uint32)
            in_maps = [{**m, self.dbg_name: dbg} for m in in_maps]
        concat = [
            np.concatenate([np.asarray(m[name]) for m in in_maps], axis=0)
            for name in self.in_names
        ]
        zeros = [
            np.zeros((n * a.shape[0], *a.shape[1:]), a.dtype) for a in self.out_avals
        ]
        outs = jax.block_until_ready(self.fn(*concat, *zeros))
        return [
            {
                name: np.asarray(outs[i]).reshape(n, *self.out_avals[i].shape)[c]
                for i, name in enumerate(self.out_names)
            }
            for c in range(n)
        ]


def _caching_run_bass_via_pjrt(nc, in_maps, n_cores):
    entry = _PJRT_CACHE.get(id(nc))
    if entry is None:
        entry = _CachedSpmd(nc, n_cores)
        _PJRT_CACHE[id(nc)] = entry
    return entry.run(in_maps)


_b2j.run_bass_via_pjrt = _caching_run_bass_via_pjrt

# ------------------------------------------------------------- constants --
B, D, H, W = 2, 8, 56, 56
CPG, G, K = 8, 8, 27
OCG = 81
V = D * H * W
PLANE = 3364  # 58*58
NB, BH = 14, 4
P = NB * CPG  # 112
CH = D * BH * W  # 1792
XD, XH, XWW = 14, 10, 62
XSZ = XD * XH * XWW
XVOL = XD * 62 * 62
SS = 5
CLAMP = 1.999
EPS = 1e-5
F32 = mybir.dt.float32
AX = mybir.AxisListType
ALU = mybir.AluOpType
ACTF = mybir.ActivationFunctionType
RG8 = [[0, 1, 2, 3, 4, 5, 6, 7]]
NCHK = 4
CSZ = B * V // NCHK  # 12544 = one batch half (7 h-bands)


def mkap(tile, off, dims):
    ap = tile[:]
    return bass.AP(tensor=ap.tensor, offset=ap.offset + off,
                   ap=[list(ap.ap[0])] + [list(d) for d in dims])


def dmkap(t_ap, off, dims):
    return bass.AP(tensor=t_ap.tensor, offset=t_ap.offset + off,
                   ap=[list(d) for d in dims])


def zero_dram(nc, zero_sb, dram_ap, parts, total):
    n512 = total // 512
    rem = total - n512 * 512
    nc.sync.dma_start(out=dmkap(dram_ap, 0, [[total, parts], [512, n512], [1, 512]]),
                      in_=dmkap(zero_sb[:], 0, [[512, parts], [0, n512], [1, 512]]))
    if rem:
        nc.sync.dma_start(out=dmkap(dram_ap, n512 * 512, [[total, parts], [1, rem]]),
                          in_=dmkap(zero_sb[:], 0, [[512, parts], [1, rem]]))


def conv_phase(nc, tc, xpad_dram, wt_sb, bias_sb, off_dram):
    """27-tap conv: xpad_dram [64, B*10*PLANE] -> off_dram [81, B*D*3136]."""
    GUARD = 64
    # row-aligned n-chunks of the padded plane: (n0, nsz, valid_row_start, n_valid)
    CHUNKS = []
    for r0 in range(0, 58, 8):
        nr = min(8, 58 - r0)
        v0 = max(1, r0)
        v1 = min(57, r0 + nr)
        CHUNKS.append((r0 * 58, nr * 58, v0 - r0, v1 - v0))
    with tc.tile_pool(name="convp", bufs=2) as pool, \
         tc.tile_pool(name="convps", bufs=4, space="PSUM") as pspool:
        for b in range(B):
            for j in range(4):
                xpc = pool.tile([64, 2 * GUARD + 4 * PLANE], F32, tag="xpc")
                nc.vector.memset(xpc[:, :GUARD], 0.0)
                nc.vector.memset(xpc[:, GUARD + 4 * PLANE:], 0.0)
                nc.sync.dma_start(
                    out=xpc[:, GUARD:GUARD + 4 * PLANE],
                    in_=dmkap(xpad_dram[:], (b * 10 + 2 * j) * PLANE,
                              [[B * 10 * PLANE, 64], [1, 4 * PLANE]]))
                for ds in range(2):
                    d = 2 * j + ds
                    for (n0, nsz, vr, nv) in CHUNKS:
                        ps = pspool.tile([OCG, 512], F32, tag="cps")
                        for k in range(K):
                            kd, kh, kw = k // 9, (k // 3) % 3, k % 3
                            roff = GUARD + (ds + kd) * PLANE + (kh - 1) * 58 + (kw - 1) + n0
                            nc.tensor.matmul(ps[:, :nsz], wt_sb[:, k, :],
                                             mkap(xpc, roff, [[1, nsz]]),
                                             start=(k == 0), stop=(k == K - 1))
                        ot = pool.tile([OCG, 512], F32, tag="convot")
                        nc.vector.tensor_tensor(
                            out=ot[:, :nsz], in0=ps[:, :nsz],
                            in1=mkap(bias_sb, 0, [[0, nsz]]), op=ALU.add)
                        if nv <= 0:
                            continue
                        # store valid interior rows to canonical [81, B, D, 56, 56]
                        real_r0 = n0 // 58 + vr - 1
                        nc.sync.dma_start(
                            out=dmkap(off_dram[:], (b * D + d) * 3136 + real_r0 * 56,
                                      [[B * D * 3136, OCG], [1, nv * 56]]),
                            in_=mkap(ot, vr * 58 + 1, [[58, nv], [1, 56]]))


def dense_phase(nc, tc, xw_dram, off_dram, wd_sb, partial_dram, colsd_dram):
    """Dense 5^3 deform + einsum -> partial_dram [64, B*V] (band-perm)."""
    with tc.tile_pool(name="densep", bufs=1) as pool, \
         tc.tile_pool(name="densew", bufs=1) as wpool, \
         tc.tile_pool(name="denseps", bufs=2, space="PSUM") as pspool:
        for b in range(B):
            xw = pool.tile([P, XSZ], F32, tag="xw")
            for dd in range(XD):
                nc.sync.dma_start(
                    out=mkap(xw, dd * XH * XWW, [[1, 620]]),
                    in_=dmkap(xw_dram[:], b * XVOL + dd * 62 * 62,
                              [[BH * XWW, NB], [B * XVOL, CPG], [1, XH * XWW]]))
            for k in range(K):
                kd, kh, kw = k // 9 - 1, (k // 3) % 3 - 1, k % 3 - 1
                offt = pool.tile([P, 3, CH], F32, tag="offt")
                for ax in range(3):
                    for dd in range(D):
                        nc.sync.dma_start(
                            out=mkap(offt, ax * CH + dd * BH * W, [[1, BH * W]]),
                            in_=dmkap(off_dram[:],
                                      (3 * k + ax) * B * D * 3136 + (b * D + dd) * 3136,
                                      [[BH * W, NB], [0, CPG], [1, BH * W]]))
                nc.vector.tensor_scalar(out=offt[:], in0=offt[:], scalar1=CLAMP,
                                        scalar2=-CLAMP, op0=ALU.min, op1=ALU.max)
                hw = pool.tile([P, SS, CH], F32, tag="hw")
                for a in range(SS):
                    nc.scalar.activation(hw[:, a, :], offt[:, 2, :], ACTF.Abs,
                                         bias=float(-(a - 2)), scale=1.0)
                    nc.scalar.activation(hw[:, a, :], hw[:, a, :], ACTF.Relu,
                                         bias=1.0, scale=-1.0)
                cols = wpool.tile([P, CH], F32, tag="cols")
                pt = wpool.tile([P, CH], F32, tag="pt")
                at = wpool.tile([P, CH], F32, tag="at")
                tt = wpool.tile([P, CH], F32, tag="tt")
                hdsl = pool.tile([P, CH], F32, tag="hdsl")
                hhsl = pool.tile([P, CH], F32, tag="hhsl")
                first = True
                for sd in range(SS):
                    nc.scalar.activation(hdsl[:], offt[:, 0, :], ACTF.Abs,
                                         bias=float(-(sd - 2)), scale=1.0)
                    nc.scalar.activation(hdsl[:], hdsl[:], ACTF.Relu,
                                         bias=1.0, scale=-1.0)
                    for sh in range(SS):
                        nc.scalar.activation(hhsl[:], offt[:, 1, :], ACTF.Abs,
                                             bias=float(-(sh - 2)), scale=1.0)
                        nc.scalar.activation(hhsl[:], hhsl[:], ACTF.Relu,
                                             bias=1.0, scale=-1.0)
                        nc.vector.tensor_tensor(out=pt[:], in0=hdsl[:],
                                                in1=hhsl[:], op=ALU.mult)
                        for sw in range(SS):
                            xoff = ((1 + kd + sd) * XH * XWW + (1 + kh + sh) * XWW
                                    + (1 + kw + sw))
                            xap = mkap(xw, xoff, [[XH * XWW, D], [XWW, BH], [1, W]])
                            dst = at if sw == 0 else tt
                            nc.vector.tensor_tensor(out=dst[:], in0=xap,
                                                    in1=hw[:, sw, :], op=ALU.mult)
                            if sw > 0:
                                nc.vector.tensor_tensor(out=at[:], in0=at[:],
                                                        in1=tt[:], op=ALU.add)
                        if first:
                            nc.vector.tensor_tensor(out=cols[:], in0=pt[:], in1=at[:],
                                                    op=ALU.mult)
                            first = False
                        else:
                            nc.gpsimd.tensor_tensor(out=tt[:], in0=pt[:], in1=at[:],
                                                    op=ALU.mult)
                            nc.gpsimd.tensor_tensor(out=cols[:], in0=cols[:], in1=tt[:],
                                                    op=ALU.add)
                nc.sync.dma_start(
                    out=dmkap(colsd_dram[:], (b * K + k) * CH,
                              [[B * K * CH, P], [1, CH]]),
                    in_=cols[:])
            tc.strict_bb_all_engine_barrier()
            # einsum: psum accumulate over taps per band
            for hb in range(NB):
                ps2 = pspool.tile([64, 2048], F32, tag="eps")
                for k in range(K):
                    cr = wpool.tile([CPG, CH], F32, tag="colsr")
                    nc.sync.dma_start(
                        out=cr[:],
                        in_=dmkap(colsd_dram[:], hb * CPG * B * K * CH + (b * K + k) * CH,
                                  [[B * K * CH, CPG], [1, CH]]))
                    for i in range(4):
                        nc.tensor.matmul(ps2[:, i * 512:i * 512 + 448], wd_sb[:, k, :],
                                         cr[:, i * 448:(i + 1) * 448],
                                         start=(k == 0), stop=(k == K - 1))
                pot = wpool.tile([64, CH], F32, tag="pot")
                nc.vector.tensor_copy(out=pot[:], in_=mkap(ps2, 0, [[512, 4], [1, 448]]))
                nc.sync.dma_start(
                    out=dmkap(partial_dram[:], b * V + hb * CH, [[B * V, 64], [1, CH]]),
                    in_=pot[:])


def ensure_consts(nc):
    for v in (2.0, -2.0, -1.0, 1e-5):
        key = (F32, v)
        if key not in nc.const_aps.aps:
            t = nc.alloc_sbuf_tensor(f"const-f32-{v}", [128, 1], F32)
            nc.gpsimd.memset(t.ap(), v)
            nc.const_aps.aps[key] = t.ap()


def bn_stats(nc, tc, pool, src_dram, gamma_sb, beta_sb, parts, pfx):
    """Training-mode BN from src_dram [parts, B*V] -> (scale, shift) [parts,1].

    Small [parts,1] tiles live in `pool`; the chunk buffers live in a
    nested scope so the SBUF is released before the apply phase.
    """
    sum_t = pool.tile([parts, 1], F32, tag=pfx + "sum")
    sq_t = pool.tile([parts, 1], F32, tag=pfx + "sq")
    t1 = pool.tile([parts, 1], F32, tag=pfx + "t1")
    t2 = pool.tile([parts, 1], F32, tag=pfx + "t2")
    with tc.tile_pool(name=pfx + "tmp", bufs=1) as tmp:
        for i in range(NCHK):
            ht = tmp.tile([parts, CSZ], F32, tag=pfx + "h")
            sqv = tmp.tile([parts, CSZ], F32, tag=pfx + "sqv")
            nc.sync.dma_start(out=ht[:],
                              in_=dmkap(src_dram, i * CSZ, [[B * V, parts], [1, CSZ]]))
            nc.vector.tensor_reduce(out=t1[:], in_=ht[:], axis=AX.X, op=ALU.add)
            nc.vector.tensor_tensor(out=sqv[:], in0=ht[:], in1=ht[:], op=ALU.mult)
            nc.vector.tensor_reduce(out=t2[:], in_=sqv[:], axis=AX.X, op=ALU.add)
            if i == 0:
                nc.vector.tensor_copy(out=sum_t[:], in_=t1[:])
                nc.vector.tensor_copy(out=sq_t[:], in_=t2[:])
            else:
                nc.vector.tensor_tensor(out=sum_t[:], in0=sum_t[:], in1=t1[:], op=ALU.add)
                nc.vector.tensor_tensor(out=sq_t[:], in0=sq_t[:], in1=t2[:], op=ALU.add)
    N = float(B * V)
    scale = pool.tile([parts, 1], F32, tag=pfx + "scale")
    shift = pool.tile([parts, 1], F32, tag=pfx + "shift")
    mean = t1
    nc.vector.tensor_scalar(out=mean[:], in0=sum_t[:], scalar1=1.0 / N, scalar2=0.0,
                            op0=ALU.mult, op1=ALU.add)
    var = t2
    nc.vector.tensor_scalar(out=var[:], in0=sq_t[:], scalar1=1.0 / N, scalar2=0.0,
                            op0=ALU.mult, op1=ALU.add)
    msq = pool.tile([parts, 1], F32, tag=pfx + "msq")
    nc.vector.tensor_tensor(out=msq[:], in0=mean[:], in1=mean[:], op=ALU.mult)
    nc.vector.tensor_tensor(out=var[:], in0=var[:], in1=msq[:], op=ALU.subtract)
    rstd = pool.tile([parts, 1], F32, tag=pfx + "rstd")
    nc.scalar.activation(out=rstd[:], in_=var[:], func=ACTF.Sqrt, bias=EPS, scale=1.0)
    nc.vector.reciprocal(out=rstd[:], in_=rstd[:])
    nc.vector.tensor_tensor(out=scale[:], in0=gamma_sb[:], in1=rstd[:], op=ALU.mult)
    nc.vector.tensor_tensor(out=shift[:], in0=mean[:], in1=scale[:], op=ALU.mult)
    nc.vector.tensor_tensor(out=shift[:], in0=beta_sb[:], in1=shift[:], op=ALU.subtract)
    return scale, shift


_STAGES = ("init", "conv1", "dense1", "red1", "bn1", "conv2", "dense2", "red2", "bn2")


def build_fused(debug_dumps=(), stop_after="bn2"):
    stop = _STAGES.index(stop_after)
    nc = bass.Bass("TRN2", target_bir_lowering=False, num_devices=8)
    ensure_consts(nc)
    xg_in = nc.declare_dram_parameter("xg", [CPG, B * V], F32, isOutput=False)
    wt1_in = nc.declare_dram_parameter("wt1", [64, K * OCG], F32, isOutput=False)
    bo1_in = nc.declare_dram_parameter("bo1", [OCG, 1], F32, isOutput=False)
    wd1_in = nc.declare_dram_parameter("wd1", [CPG, K * 64], F32, isOutput=False)
    wt2_in = nc.declare_dram_parameter("wt2", [64, K * OCG], F32, isOutput=False)
    bo2_in = nc.declare_dram_parameter("bo2", [OCG, 1], F32, isOutput=False)
    wd2_in = nc.declare_dram_parameter("wd2", [CPG, K * 64], F32, isOutput=False)
    g1_in = nc.declare_dram_parameter("g1", [64, 1], F32, isOutput=False)
    b1_in = nc.declare_dram_parameter("b1", [64, 1], F32, isOutput=False)
    g1g_in = nc.declare_dram_parameter("g1g", [CPG, 1], F32, isOutput=False)
    b1g_in = nc.declare_dram_parameter("b1g", [CPG, 1], F32, isOutput=False)
    g2g_in = nc.declare_dram_parameter("g2g", [CPG, 1], F32, isOutput=False)
    b2g_in = nc.declare_dram_parameter("b2g", [CPG, 1], F32, isOutput=False)
    out_dram = nc.declare_dram_parameter("outg", [CPG, B * V], F32, isOutput=True)

    stage = nc.dram_tensor("stage_s", [CPG, B * V], F32)
    xfull = nc.dram_tensor("xfull_s", [64, B * V], F32)
    xw_dram = nc.dram_tensor("xw_s", [CPG, B * XVOL], F32)
    xpad = nc.dram_tensor("xpad_s", [64, B * 10 * PLANE], F32)
    off_dram = nc.dram_tensor("off_s", [OCG, B * D * 3136], F32)
    colsd = nc.dram_tensor("colsd_s", [P, B * K * CH], F32)
    part1 = nc.dram_tensor("part1_s", [64, B * V], F32)
    h1sh = nc.dram_tensor("h1sh_s", [64, B * V], F32)
    h1s = nc.dram_tensor("h1s_s", [CPG, B * V], F32)
    part2 = nc.dram_tensor("part2_s", [64, B * V], F32)
    h2s = nc.dram_tensor("h2s_s", [CPG, B * V], F32)

    dump_outs = {}
    for dn, tens, shape in (
        ("xfull", "xfull", [64, B * V]),
        ("xpad1", "xpad", [64, B * 10 * PLANE]),
        ("xw1", "xw", [CPG, B * XVOL]),
        ("off1", "off", [OCG, B * D * 3136]),
        ("part1", "part1", [64, B * V]),
        ("h1sh", "h1sh", [64, B * V]),
        ("h1s", "h1s", [CPG, B * V]),
        ("xpad2", "xpad", [64, B * 10 * PLANE]),
        ("xw2", "xw", [CPG, B * XVOL]),
        ("off2", "off", [OCG, B * D * 3136]),
        ("part2", "part2", [64, B * V]),
        ("h2s", "h2s", [CPG, B * V]),
    ):
        if dn in debug_dumps:
            dump_outs[dn] = nc.declare_dram_parameter(
                dn + "_d", shape, F32, isOutput=True)

    def dump(tc, dn, src):
        if dn not in dump_outs:
            return
        dst = dump_outs[dn]
        shape = [int(s) for s in dst.shape]
        nc.sync.dma_start(
            out=dmkap(dst[:], 0, [[shape[1], shape[0]], [1, shape[1]]]),
            in_=dmkap(src[:], 0, [[shape[1], shape[0]], [1, shape[1]]]))
        tc.strict_bb_all_engine_barrier()

    with TileContext(nc) as tc:
        with tc.tile_pool(name="single", bufs=1) as sp:
            wt1_sb = sp.tile([64, K, OCG], F32, tag="wt1")
            nc.sync.dma_start(out=wt1_sb[:], in_=wt1_in[:].rearrange("p (k o) -> p k o", k=K))
            bo1_sb = sp.tile([OCG, 1], F32, tag="bo1")
            nc.sync.dma_start(out=bo1_sb[:], in_=bo1_in[:])
            wd1_sb = sp.tile([CPG, K, 64], F32, tag="wd1")
            nc.sync.dma_start(out=wd1_sb[:], in_=wd1_in[:].rearrange("p (k o) -> p k o", k=K))
            wt2_sb = sp.tile([64, K, OCG], F32, tag="wt2")
            nc.sync.dma_start(out=wt2_sb[:], in_=wt2_in[:].rearrange("p (k o) -> p k o", k=K))
            bo2_sb = sp.tile([OCG, 1], F32, tag="bo2")
            nc.sync.dma_start(out=bo2_sb[:], in_=bo2_in[:])
            wd2_sb = sp.tile([CPG, K, 64], F32, tag="wd2")
            nc.sync.dma_start(out=wd2_sb[:], in_=wd2_in[:].rearrange("p (k o) -> p k o", k=K))
            g1_sb = sp.tile([64, 1], F32, tag="g1")
            nc.sync.dma_start(out=g1_sb[:], in_=g1_in[:])
            b1_sb = sp.tile([64, 1], F32, tag="b1")
            nc.sync.dma_start(out=b1_sb[:], in_=b1_in[:])
            g1g_sb = sp.tile([CPG, 1], F32, tag="g1g")
            nc.sync.dma_start(out=g1g_sb[:], in_=g1g_in[:])
            b1g_sb = sp.tile([CPG, 1], F32, tag="b1g")
            nc.sync.dma_start(out=b1g_sb[:], in_=b1g_in[:])
            g2g_sb = sp.tile([CPG, 1], F32, tag="g2g")
            nc.sync.dma_start(out=g2g_sb[:], in_=g2g_in[:])
            b2g_sb = sp.tile([CPG, 1], F32, tag="b2g")
            nc.sync.dma_start(out=b2g_sb[:], in_=b2g_in[:])

            # stage x for the AllGather; zero pad buffers; xw interior <- xg
            with tc.tile_pool(name="initp", bufs=1) as ip:
                zt = ip.tile([64, 512], F32, tag="zt")
                nc.vector.memset(zt[:], 0.0)
                zero_dram(nc, zt, xpad[:], 64, B * 10 * PLANE)
                zero_dram(nc, zt, xw_dram[:], CPG, B * XVOL)
                nc.sync.dma_start(out=stage[:], in_=xg_in[:])
                # raw DRAM tensors are not dep-tracked: order the zero fill
                # before the interior fills explicitly
                tc.strict_bb_all_engine_barrier()
                for b in range(B):
                    for d in range(D):
                        nc.sync.dma_start(
                            out=dmkap(xw_dram[:], b * XVOL + (d + 3) * 3844 + 3 * 62 + 3,
                                      [[B * XVOL, CPG], [62, 56], [1, 56]]),
                            in_=dmkap(xg_in[:], b * V + d * 3136,
                                      [[B * V, CPG], [56, 56], [1, 56]]))
            tc.strict_bb_all_engine_barrier()
            nc.gpsimd.collective_compute(
                "AllGather", ALU.bypass, replica_groups=RG8,
                ins=[stage[:].opt()], outs=[xfull[:].opt()])
            tc.strict_bb_all_engine_barrier()
            # xpad interior <- xfull
            for b in range(B):
                for d in range(D):
                    nc.sync.dma_start(
                        out=dmkap(xpad[:], (b * 10 + d + 1) * PLANE + 59,
                                  [[B * 10 * PLANE, 64], [58, 56], [1, 56]]),
                        in_=dmkap(xfull[:], b * V + d * 3136,
                                  [[B * V, 64], [56, 56], [1, 56]]))
            tc.strict_bb_all_engine_barrier()
            dump(tc, "xfull", xfull)
            dump(tc, "xpad1", xpad)
            dump(tc, "xw1", xw_dram)
            if stop <= 0:
                return nc

            # ---- layer 1
            conv_phase(nc, tc, xpad, wt1_sb, bo1_sb, off_dram)
            tc.strict_bb_all_engine_barrier()
            dump(tc, "off1", off_dram)
            if stop <= 1:
                return nc
            dense_phase(nc, tc, xw_dram, off_dram, wd1_sb, part1, colsd)
            tc.strict_bb_all_engine_barrier()
            dump(tc, "part1", part1)
            if stop <= 2:
                return nc
            nc.gpsimd.collective_compute(
                "AllReduce", ALU.add, replica_groups=RG8,
                ins=[part1[:].opt()], outs=[h1sh[:].opt()])
            nc.gpsimd.collective_compute(
                "ReduceScatter", ALU.add, replica_groups=RG8,
                ins=[part1[:].opt()], outs=[h1s[:].opt()])
            tc.strict_bb_all_engine_barrier()
            dump(tc, "h1sh", h1sh)
            dump(tc, "h1s", h1s)
            if stop <= 3:
                return nc

            # ---- BN1 + relu, written straight into xpad / xw interiors
            sc64, sh64 = bn_stats(nc, tc, sp, h1sh[:], g1_sb, b1_sb, 64, "bna")
            sc8, sh8 = bn_stats(nc, tc, sp, h1s[:], g1g_sb, b1g_sb, CPG, "bnb")
            with tc.tile_pool(name="bnp1", bufs=1) as bnpool:
                for i in range(NCHK):
                    b, half = i // 2, i % 2
                    ht = bnpool.tile([64, CSZ], F32, tag="bnah")
                    nc.sync.dma_start(out=ht[:],
                                      in_=dmkap(h1sh[:], i * CSZ, [[B * V, 64], [1, CSZ]]))
                    nc.scalar.activation(out=ht[:], in_=ht[:], func=ACTF.Relu,
                                         bias=sh64[:], scale=sc64[:])
                    htg = bnpool.tile([CPG, CSZ], F32, tag="bnbh")
                    nc.sync.dma_start(out=htg[:],
                                      in_=dmkap(h1s[:], i * CSZ, [[B * V, CPG], [1, CSZ]]))
                    nc.scalar.activation(out=htg[:], in_=htg[:], func=ACTF.Relu,
                                         bias=sh8[:], scale=sc8[:])
                    for hbr in range(7):
                        hb = half * 7 + hbr
                        for r in range(BH):
                            nc.sync.dma_start(
                                out=dmkap(xpad[:],
                                          (b * 10 + 1) * PLANE + (4 * hb + r + 1) * 58 + 1,
                                          [[B * 10 * PLANE, 64], [PLANE, D], [1, 56]]),
                                in_=mkap(ht, hbr * CH + r * W, [[BH * W, D], [1, 56]]))
                            nc.sync.dma_start(
                                out=dmkap(xw_dram[:],
                                          b * XVOL + 3 * 3844 + (4 * hb + r + 3) * 62 + 3,
                                          [[B * XVOL, CPG], [3844, D], [1, 56]]),
                                in_=mkap(htg, hbr * CH + r * W, [[BH * W, D], [1, 56]]))
            tc.strict_bb_all_engine_barrier()

            # ---- layer 2
            dump(tc, "xpad2", xpad)
            dump(tc, "xw2", xw_dram)
            if stop <= 4:
                return nc
            conv_phase(nc, tc, xpad, wt2_sb, bo2_sb, off_dram)
            tc.strict_bb_all_engine_barrier()
            dump(tc, "off2", off_dram)
            if stop <= 5:
                return nc
            dense_phase(nc, tc, xw_dram, off_dram, wd2_sb, part2, colsd)
            tc.strict_bb_all_engine_barrier()
            dump(tc, "part2", part2)
            if stop <= 6:
                return nc
            nc.gpsimd.collective_compute(
                "ReduceScatter", ALU.add, replica_groups=RG8,
                ins=[part2[:].opt()], outs=[h2s[:].opt()])
            tc.strict_bb_all_engine_barrier()
            dump(tc, "h2s", h2s)
            if stop <= 7:
                return nc

            # ---- BN2 + residual + relu, canonical per-group output
            sc2, sh2 = bn_stats(nc, tc, sp, h2s[:], g2g_sb, b2g_sb, CPG, "bnc")
            with tc.tile_pool(name="bnp2", bufs=1) as bnpool:
                for i in range(NCHK):
                    b, half = i // 2, i % 2
                    ht = bnpool.tile([CPG, CSZ], F32, tag="bnch")
                    rt = bnpool.tile([CPG, CSZ], F32, tag="bncr")
                    nc.sync.dma_start(out=ht[:],
                                      in_=dmkap(h2s[:], i * CSZ, [[B * V, CPG], [1, CSZ]]))
                    # residual read canonical -> band-perm order
                    for hbr in range(7):
                        hb = half * 7 + hbr
                        nc.sync.dma_start(
                            out=mkap(rt, hbr * CH, [[BH * W, D], [1, BH * W]]),
                            in_=dmkap(xg_in[:], b * V + hb * BH * W,
                                      [[B * V, CPG], [H * W, D], [1, BH * W]]))
                    nc.vector.tensor_tensor(out=ht[:], in0=ht[:],
                                            in1=mkap(sc2, 0, [[0, CSZ]]), op=ALU.mult)
                    nc.vector.tensor_tensor(out=ht[:], in0=ht[:],
                                            in1=mkap(sh2, 0, [[0, CSZ]]), op=ALU.add)
                    nc.vector.tensor_tensor(out=ht[:], in0=ht[:], in1=rt[:], op=ALU.add)
                    nc.vector.tensor_scalar(out=ht[:], in0=ht[:], scalar1=0.0,
                                            scalar2=0.0, op0=ALU.max, op1=ALU.add)
                    # write canonical: band-perm chunk -> (d, h, w)
                    for hbr in range(7):
                        hb = half * 7 + hbr
                        nc.sync.dma_start(
                            out=dmkap(out_dram[:], b * V + hb * BH * W,
                                      [[B * V, CPG], [H * W, D], [1, BH * W]]),
                            in_=mkap(ht, hbr * CH, [[BH * W, D], [1, BH * W]]))
    return nc


_FUSED_NC = None


def kernel(**inputs):
    global _FUSED_NC
    x = np.ascontiguousarray(inputs["x"], dtype=np.float32)
    xt = x.transpose(1, 0, 2, 3, 4).reshape(64, B * V)  # [64, B*V] canonical

    def wslices(w_off, b_off, w_dc):
        wts, bs, wds = [], [], []
        w_off = np.asarray(w_off, np.float32).reshape(G * OCG, 64, K)
        w_dc = np.asarray(w_dc, np.float32).reshape(64, G, CPG, K)
        b_off = np.asarray(b_off, np.float32)
        for g in range(G):
            wts.append(np.ascontiguousarray(
                w_off[g * OCG:(g + 1) * OCG].transpose(1, 2, 0)).reshape(64, K * OCG))
            bs.append(np.ascontiguousarray(b_off[g * OCG:(g + 1) * OCG]).reshape(OCG, 1))
            wds.append(np.ascontiguousarray(
                w_dc[:, g].transpose(1, 2, 0)).reshape(CPG, K * 64))
        return wts, bs, wds

    wt1, bo1, wd1 = wslices(inputs["w_off1"], inputs["b_off1"], inputs["w_dc1"])
    wt2, bo2, wd2 = wslices(inputs["w_off2"], inputs["b_off2"], inputs["w_dc2"])
    g1 = np.asarray(inputs["gamma1"], np.float32).reshape(64, 1)
    b1 = np.asarray(inputs["beta1"], np.float32).reshape(64, 1)
    g2 = np.asarray(inputs["gamma2"], np.float32).reshape(64, 1)
    b2 = np.asarray(inputs["beta2"], np.float32).reshape(64, 1)

    if _FUSED_NC is None:
        _FUSED_NC = build_fused()

    in_maps = [{
        "xg": np.ascontiguousarray(xt[g * CPG:(g + 1) * CPG]),
        "wt1": wt1[g], "bo1": bo1[g], "wd1": wd1[g],
        "wt2": wt2[g], "bo2": bo2[g], "wd2": wd2[g],
        "g1": g1, "b1": b1,
        "g1g": np.ascontiguousarray(g1[g * CPG:(g + 1) * CPG]),
        "b1g": np.ascontiguousarray(b1[g * CPG:(g + 1) * CPG]),
        "g2g": np.ascontiguousarray(g2[g * CPG:(g + 1) * CPG]),
        "b2g": np.ascontiguousarray(b2[g * CPG:(g + 1) * CPG]),
    } for g in range(G)]
    res = bass_utils.run_bass_kernel_spmd(_FUSED_NC, in_maps,
                                          core_ids=list(range(8))).results
    out = np.concatenate([res[g]["outg"] for g in range(G)], axis=0)  # [64, B*V]
    out = out.reshape(64, B, D, H, W).transpose(1, 0, 2, 3, 4)
    return np.ascontiguousarray(out)
